# revision 17
# baseline (speedup 1.0000x reference)
"""Trainium2 Bass kernel for nn_FFTConv: y = tanh(conv_circ(u, k) + D*u).

v5 design — direct block-Toeplitz FIR (no FFT):
  The SSM kernel k[h,t] = Re(sum_p BC[h,p] * A_p^t) decays like r_max^t,
  so the 8192-tap circular convolution is numerically a short FIR. The
  number of 128-tap history blocks C is chosen at runtime from the pole
  radii (C=1 for the reference instance, r_max=0.956 -> truncation
  ~1e-5); the conv is computed on the PE as NST=C+1 accumulating
  128x128 block-Toeplitz stationaries per channel:

     y[j + 128 m] = sum_{c=0}^{C} sum_lam M_c[lam, j] * u[lam, m - c]
     M_c[lam, j]  = k[128 c + j - lam]   (D folded into k[0])

  Per h (16 batch rows x 8192): one contiguous DMA in (bf16, wrap
  blocks pre-appended on host), 2*NST matmuls (free-512), a fused
  tanh+downcast on Act, an int8 quantize (x127) on the otherwise-idle
  DVE, one int8 DMA out (dequantized to f32 on host). H-sharded over
  8 cores (32 ch/core).

  Engine budget per core at C=1: PE ~27us, Act ~33us, DVE ~19us,
  DMA ~36us (in 8.5MB bf16 + out 4.2MB int8).
"""
import os
import sys
import numpy as np

for p in ("/opt/trn_rl_repo", "/root/.axon_site/_ro/trn_rl_repo"):
    if os.path.isdir(p) and p not in sys.path:
        sys.path.append(p)

import ml_dtypes

BF16 = ml_dtypes.bfloat16

B, H, L, P = 16, 256, 8192, 64
NCORES = 8
HSH = H // NCORES          # 32 channels per core
MB = L // 128              # 64 output blocks per row
REPEAT = int(os.environ.get("KERNEL_REPEAT", "1"))
IOBUFS = int(os.environ.get("KERNEL_IOBUFS", "4"))
PFBUFS = int(os.environ.get("KERNEL_PFBUFS", "3"))
ACTSPLIT = int(os.environ.get("KERNEL_ACTSPLIT", "1"))  # tanh calls per h
OUTQ = os.environ.get("KERNEL_OUTQ", "sync")            # queue for output DMA
NODMA = os.environ.get("KERNEL_NODMA", "0") == "1"      # timing probe: skip io DMA
OUT8 = os.environ.get("KERNEL_OUT8", "1") == "1"        # int8 output path
GRP = int(os.environ.get("KERNEL_GRP", "4"))            # channels per DMA batch
YSCALE = 127.0

# History-block count. None = auto from pole radii (set by _choose_c).
_C_ENV = os.environ.get("KERNEL_C", "")
C = int(_C_ENV) if _C_ENV else None

_CACHE = {}


def _choose_c(A_re, A_im):
    """Smallest C with truncation error far below the bf16 noise floor."""
    global C
    if _C_ENV:
        C = int(_C_ENV)
        return C
    r = float(np.max(np.hypot(np.asarray(A_re, np.float64),
                              np.asarray(A_im, np.float64))))
    r = min(max(r, 1e-6), 0.9999)
    for c in range(1, 6):
        if r ** (128 * (c + 1)) < 1e-4:
            break
    C = c
    return C


def _build(nc_mod, c_blocks):
    bass, tile, mybir, bacc = nc_mod
    dt = mybir.dt
    f32 = dt.float32
    bf16 = dt.bfloat16
    nst = c_blocks + 1
    mbe = MB + c_blocks

    nc = bacc.Bacc("TRN2", target_bir_lowering=False, debug=False)
    AF = mybir.ActivationFunctionType
    OP = mybir.AluOpType

    # u2: [lam, h, (m' b)] host-relayouted with C wrap blocks prepended.
    # y2: [j, h, (m b)]. kt: stationaries [lam, h, c, j].
    u_d = nc.declare_dram_parameter("u2_sh", [128, HSH, mbe * B], bf16, isOutput=False)
    ydt = dt.int8 if OUT8 else bf16
    y_d = nc.declare_dram_parameter("y2_sh", [128, HSH, MB * B], ydt, isOutput=True)
    kt_d = nc.declare_dram_parameter("kt_sh", [128, HSH, nst, 128], bf16, isOutput=False)

    with tile.TileContext(nc) as tc:
        with tc.tile_pool(name="const", bufs=1) as cpool:
            kt = cpool.tile([128, HSH, nst, 128], bf16, tag="kt")
            nc.sync.dma_start(
                kt[:].rearrange("p a b c -> p (a b c)"),
                kt_d[:].rearrange("p a b c -> p (a b c)"))

            with tc.tile_pool(name="io", bufs=IOBUFS) as iop, \
                 tc.tile_pool(name="pf", bufs=PFBUFS, space=bass.MemorySpace.PSUM) as pfp:

                for _rep in range(REPEAT):
                    for hg in range(0, HSH, GRP):
                        uc = iop.tile([128, GRP, mbe * B], bf16, tag="uc", name="uc")
                        if NODMA:
                            nc.gpsimd.memset(uc[:].rearrange("p a b -> p (a b)"), 0)
                        else:
                            nc.sync.dma_start(
                                uc[:].rearrange("p a b -> p (a b)"),
                                u_d[:, hg:hg + GRP].rearrange("p a b -> p (a b)"))

                        odt = dt.int8 if OUT8 else bf16
                        ot = iop.tile([128, GRP, MB * B], odt, tag="ot", name="ot")
                        for hi in range(GRP):
                            h = hg + hi
                            Y = pfp.tile([128, MB * B], f32, tag="Y", name="Y")
                            for half in range(2):
                                ob = half * (MB // 2) * B            # 512
                                for cc in range(nst):
                                    mv = (c_blocks - cc + half * (MB // 2)) * B
                                    nc.tensor.matmul(
                                        Y[:, ob:ob + 512],
                                        kt[:, h, cc, :],
                                        uc[:, hi, mv:mv + 512],
                                        start=(cc == 0), stop=(cc == nst - 1))

                            if OUT8:
                                yo = iop.tile([128, MB * B], bf16, tag="yo", name="yo")
                                nc.scalar.activation(yo[:], Y[:], AF.Tanh)
                                nc.vector.tensor_scalar_mul(ot[:, hi], yo[:], YSCALE)
                            else:
                                ns = (MB * B) // ACTSPLIT
                                for s in range(ACTSPLIT):
                                    nc.scalar.activation(
                                        ot[:, hi, s * ns:(s + 1) * ns],
                                        Y[:, s * ns:(s + 1) * ns], AF.Tanh)
                        if not NODMA:
                            outq = getattr(nc, OUTQ, nc.sync)
                            outq.dma_start(
                                y_d[:, hg:hg + GRP].rearrange("p a b -> p (a b)"),
                                ot[:].rearrange("p a b -> p (a b)"))

    nc.compile()
    return nc


def _get_program():
    assert C is not None, "call _choose_c/make_in_maps/kernel first"
    key = ("prog", REPEAT, IOBUFS, PFBUFS, ACTSPLIT, OUTQ, C, NODMA, OUT8, GRP)
    if key not in _CACHE:
        import concourse.bass as bass
        import concourse.tile as tile
        from concourse import mybir, bacc
        _CACHE[key] = _build((bass, tile, mybir, bacc), C)
    return _CACHE[key]


def make_in_maps(u, A_re, A_im, BC_re, BC_im, D):
    u = np.asarray(u, dtype=np.float32)
    A_re = np.asarray(A_re, dtype=np.float64)
    A_im = np.asarray(A_im, dtype=np.float64)
    BC_re = np.asarray(BC_re, dtype=np.float64)
    BC_im = np.asarray(BC_im, dtype=np.float64)
    D = np.asarray(D, dtype=np.float64)
    _choose_c(A_re, A_im)
    nst = C + 1
    nt = 128 * nst
    mbe = MB + C

    # FIR taps k[h,t] = Re(sum_p BC A^t), t in [0, nt); D folded into k[0].
    A = A_re + 1j * A_im
    t = np.arange(nt)
    V = np.exp(np.log(A)[:, None] * t[None, :])            # (P, nt)
    kr = np.real((BC_re + 1j * BC_im) @ V)                  # (H, nt)
    kr[:, 0] += D

    # Toeplitz stationaries M[h, c, lam, j] = k[128c + j - lam]
    kr_pad = np.concatenate([np.zeros((H, 127)), kr], axis=1)
    lam = np.arange(128)
    j = np.arange(128)
    cs = np.arange(nst)
    idx = 128 * cs[:, None, None] + j[None, None, :] - lam[None, :, None] + 127
    M = kr_pad[:, idx].astype(BF16)                         # (H, nst, lam, j)

    ub = u.astype(BF16).reshape(B, H, MB, 128)
    in_maps = []
    for core in range(NCORES):
        hs = slice(core * HSH, (core + 1) * HSH)
        v = ub[:, hs].transpose(3, 1, 2, 0)                 # [lam, h, m, b]
        ue = np.concatenate([v[:, :, MB - C:, :], v], axis=2)
        in_maps.append({
            "u2_sh": np.ascontiguousarray(ue.reshape(128, HSH, mbe * B)),
            "kt_sh": np.ascontiguousarray(M[hs].transpose(2, 0, 1, 3)),
        })
    return in_maps


def kernel(u, A_re, A_im, BC_re, BC_im, D):
    from concourse.bass_utils import run_bass_kernel_spmd

    in_maps = make_in_maps(u, A_re, A_im, BC_re, BC_im, D)
    nc = _get_program()

    res = None
    last_err = None
    for attempt in range(3):
        try:
            res = run_bass_kernel_spmd(nc, in_maps, list(range(NCORES)))
            break
        except Exception as e:  # transient NRT_EXEC_UNIT_UNRECOVERABLE flakes
            last_err = e
            import time as _time
            _time.sleep(2.0)
    if res is None:
        raise last_err
    outs = []
    for core in range(NCORES):
        y2 = res.results[core]["y2_sh"].reshape(128, HSH, MB, B)
        y2 = y2.transpose(3, 1, 2, 0).reshape(B, HSH, L)
        outs.append(y2)
    out = np.concatenate(outs, axis=1).astype(np.float32)
    if OUT8:
        out /= YSCALE
    return np.ascontiguousarray(out)


if __name__ == "__main__":
    rng = np.random.default_rng(0)
    u = rng.standard_normal((B, H, L), dtype=np.float32)
    A_re = rng.uniform(0.5, 0.99, P).astype(np.float32)
    A_im = rng.uniform(-0.5, 0.5, P).astype(np.float32)
    BC_re = rng.standard_normal((H, P), dtype=np.float32)
    BC_im = rng.standard_normal((H, P), dtype=np.float32)
    D = rng.uniform(0, 1, H).astype(np.float32)
    y = kernel(u=u, A_re=A_re, A_im=A_im, BC_re=BC_re, BC_im=BC_im, D=D)
    print("out", y.shape, y.dtype)
